# revision 6
# baseline (speedup 1.0000x reference)
"""Trainium2 Bass kernel for nn_AttentionBlock (GroupNorm -> QKV -> MHA -> proj -> residual).

Full inputs in, full output out. Sharding: 8 cores = 4 batches x 2 head-pairs.
Each core computes GroupNorm(x_b), its 2 heads' q/k/v projections, attention
(k-major scores, exp on ACT, ones-matmul denominators), and a partial output
projection over its 2 heads. Host sums the two partials per batch and adds
bias + residual.

Self-contained: hardcodes shapes from the problem spec
(x: (4, 512, 64, 64) fp32, weights 512x512, 4 heads, 32 groups, eps 1e-5).
"""
import sys
import numpy as np

if '/opt/trn_rl_repo' not in sys.path:
    sys.path.insert(0, '/opt/trn_rl_repo')

N_CORES = 8
B, C, H, W = 4, 512, 64, 64
HW = H * W            # 4096
NH, NG, EPS = 4, 32, 1e-5
HD = C // NH          # 128 head dim
SCALE = 1.0 / float(np.sqrt(HD))
NKC = HW // 128       # 32 k-chunks of 128
NQG = HW // 512       # 8 q-groups of 512
PT_BUFS = 34          # pT sliding window: 2 groups (32 tiles) + margin


def _build_program():
    import concourse.bacc as bacc
    import concourse.bass as bass
    import concourse.tile as tile
    import concourse.mybir as mybir
    from concourse import masks

    f32, f32r, bf16 = mybir.dt.float32, mybir.dt.float32r, mybir.dt.bfloat16
    AF = mybir.ActivationFunctionType
    OP = mybir.AluOpType

    nc = bacc.Bacc("TRN2", target_bir_lowering=False, debug=False, num_devices=1)

    x_d = nc.dram_tensor("x", [C, HW], f32r, kind="ExternalInput").ap()
    wqT_d = nc.dram_tensor("wqT", [C, 2 * HD], f32r, kind="ExternalInput").ap()
    wkT_d = nc.dram_tensor("wkT", [C, 2 * HD], f32r, kind="ExternalInput").ap()
    wvT_d = nc.dram_tensor("wvT", [C, 2 * HD], f32r, kind="ExternalInput").ap()
    wpT_d = nc.dram_tensor("wpT", [2 * HD, C], f32r, kind="ExternalInput").ap()
    gnw_d = nc.dram_tensor("gnw", [4, 128, 1], f32, kind="ExternalInput").ap()
    gnb_d = nc.dram_tensor("gnb", [4, 128, 1], f32, kind="ExternalInput").ap()
    bq_d = nc.dram_tensor("bq", [2, 128, 1], f32, kind="ExternalInput").ap()
    bk_d = nc.dram_tensor("bk", [2, 128, 1], f32, kind="ExternalInput").ap()
    bv_d = nc.dram_tensor("bv", [2, 128, 1], f32, kind="ExternalInput").ap()
    gt_d = nc.dram_tensor("gt", [C, NG], f32, kind="ExternalInput").ap()
    ex_d = nc.dram_tensor("ex", [NG, 4, 128], f32, kind="ExternalInput").ap()
    out_d = nc.dram_tensor("out_part", [C, HW], f32, kind="ExternalOutput").ap()

    with tile.TileContext(nc) as tc:
        with tc.tile_pool(name="consts", bufs=1) as consts, \
             tc.tile_pool(name="persist", bufs=1) as persist:
            ident_bf = consts.tile([128, 128], bf16)
            masks.make_identity(nc, ident_bf)
            ones_bf = consts.tile([128, 128], bf16)
            nc.vector.memset(ones_bf, 1.0)
            eps_t = consts.tile([128, 1], f32)
            nc.vector.memset(eps_t, EPS)
            wp_t = consts.tile([128, 2, C], f32r)
            nc.sync.dma_start(out=wp_t, in_=wpT_d.rearrange("(t p) c -> p t c", p=128))

            # Persistent per-head activations
            q_sb = [persist.tile([128, HW], f32r, tag=f"q{h}", name=f"q{h}") for h in range(2)]
            k_sb = [persist.tile([128, HW], f32r, tag=f"k{h}", name=f"k{h}") for h in range(2)]
            vT_sb = [persist.tile([128, NKC, 128], bf16, tag=f"vT{h}", name=f"vT{h}") for h in range(2)]

            # ---------------- Phase A: load x, GroupNorm, QKV, vT ----------------
            with tc.tile_pool(name="aw", bufs=1) as aw, \
                 tc.tile_pool(name="xp", bufs=4) as xp, \
                 tc.tile_pool(name="vp", bufs=1) as vp, \
                 tc.tile_pool(name="small", bufs=6) as small, \
                 tc.tile_pool(name="psA", bufs=3, space="PSUM") as psA, \
                 tc.tile_pool(name="psAs", bufs=2, space="PSUM") as psAs, \
                 tc.tile_pool(name="psG", bufs=1, space="PSUM") as psG:

                wq_t = aw.tile([128, 4, 2 * HD], f32r, tag="wq")
                wk_t = aw.tile([128, 4, 2 * HD], f32r, tag="wk")
                wv_t = aw.tile([128, 4, 2 * HD], f32r, tag="wv")
                nc.sync.dma_start(out=wq_t, in_=wqT_d.rearrange("(t p) c -> p t c", p=128))
                nc.sync.dma_start(out=wk_t, in_=wkT_d.rearrange("(t p) c -> p t c", p=128))
                nc.sync.dma_start(out=wv_t, in_=wvT_d.rearrange("(t p) c -> p t c", p=128))
                gnw_t = aw.tile([128, 4, 1], f32, tag="gnw")
                gnb_t = aw.tile([128, 4, 1], f32, tag="gnb")
                nc.sync.dma_start(out=gnw_t, in_=gnw_d.rearrange("t p one -> p t one"))
                nc.sync.dma_start(out=gnb_t, in_=gnb_d.rearrange("t p one -> p t one"))
                bq_t = aw.tile([128, 2, 1], f32, tag="bq")
                bk_t = aw.tile([128, 2, 1], f32, tag="bk")
                bv_t = aw.tile([128, 2, 1], f32, tag="bv")
                nc.sync.dma_start(out=bq_t, in_=bq_d.rearrange("h p one -> p h one"))
                nc.sync.dma_start(out=bk_t, in_=bk_d.rearrange("h p one -> p h one"))
                nc.sync.dma_start(out=bv_t, in_=bv_d.rearrange("h p one -> p h one"))
                gt_t = aw.tile([128, 4, NG], f32, tag="gt")
                nc.sync.dma_start(out=gt_t, in_=gt_d.rearrange("(t p) g -> p t g", p=128))
                ex_t = aw.tile([NG, 4, 128], f32, tag="ex")
                nc.sync.dma_start(out=ex_t, in_=ex_d)

                # load x + per-channel stats
                x_t = []
                me_t = []
                for t in range(4):
                    xt = xp.tile([128, HW], f32r, tag="x")
                    nc.sync.dma_start(out=xt, in_=x_d[t * 128:(t + 1) * 128, :])
                    x_t.append(xt)
                    xtf = xt.bitcast(f32)
                    st = small.tile([128, 8, 6], f32, tag="bnst")
                    for chk in range(8):
                        nc.vector.bn_stats(out=st[:, chk, :],
                                           in_=xtf[:, chk * 512:(chk + 1) * 512])
                    mv = small.tile([128, 2], f32, tag="mv")
                    nc.vector.bn_aggr(out=mv, in_=st)
                    # me = [mean, E[x^2]] = [mean, var + mean^2]
                    me = small.tile([128, 2], f32, tag="me")
                    m2 = small.tile([128, 1], f32, tag="m2")
                    nc.vector.tensor_copy(me[:, 0:1], mv[:, 0:1])
                    nc.vector.tensor_mul(m2, mv[:, 0:1], mv[:, 0:1])
                    nc.vector.tensor_add(me[:, 1:2], mv[:, 1:2], m2)
                    me_t.append(me)

                # group stats: psum[g, 2] = sum_c GT[c, g]/16 * me[c, :]
                gps = psG.tile([NG, 2], f32)
                for t in range(4):
                    nc.tensor.matmul(gps, gt_t[:, t, :], me_t[t],
                                     start=(t == 0), stop=(t == 3))
                gsb = small.tile([NG, 2], f32, tag="gsb")
                nc.vector.tensor_copy(gsb, gps)
                m2g = small.tile([NG, 1], f32, tag="m2g")
                var_g = small.tile([NG, 1], f32, tag="varg")
                nc.vector.tensor_mul(m2g, gsb[:, 0:1], gsb[:, 0:1])
                nc.vector.tensor_sub(var_g, gsb[:, 1:2], m2g)
                sd_g = small.tile([NG, 1], f32, tag="sdg")
                nc.scalar.activation(out=sd_g, in_=var_g, func=AF.Sqrt,
                                     bias=eps_t[0:NG, :], scale=1.0)
                rstd_g = small.tile([NG, 1], f32, tag="rstdg")
                nc.vector.reciprocal(rstd_g, sd_g)
                grp = small.tile([NG, 2], f32, tag="grp")
                nc.vector.tensor_copy(grp[:, 0:1], gsb[:, 0:1])
                nc.vector.tensor_copy(grp[:, 1:2], rstd_g)

                # per-tile affine + normalize in place (write f32r view)
                xr_t = []
                for t in range(4):
                    bcp = psG.tile([128, 2], f32, tag="bcp")
                    nc.tensor.matmul(bcp, ex_t[:, t, :], grp, start=True, stop=True)
                    bc = small.tile([128, 2], f32, tag="bc")
                    nc.vector.tensor_copy(bc, bcp)
                    A_t = small.tile([128, 1], f32, tag="At")
                    tmp = small.tile([128, 1], f32, tag="tmp")
                    B_t = small.tile([128, 1], f32, tag="Bt")
                    nc.vector.tensor_mul(A_t, bc[:, 1:2], gnw_t[:, t, :])
                    nc.vector.tensor_mul(tmp, bc[:, 0:1], A_t)
                    nc.vector.tensor_sub(B_t, gnb_t[:, t, :], tmp)
                    nc.vector.tensor_scalar(out=x_t[t], in0=x_t[t].bitcast(f32),
                                            scalar1=A_t, scalar2=B_t,
                                            op0=OP.mult, op1=OP.add)
                    xr_t.append(x_t[t])

                # QKV projections
                v_sb = [vp.tile([128, HW], bf16, tag=f"v{h}", name=f"v{h}") for h in range(2)]
                for h in range(2):
                    for wt, bt, osb, in ((wq_t, bq_t, q_sb[h]),
                                         (wk_t, bk_t, k_sb[h]),
                                         (wv_t, bv_t, v_sb[h])):
                        for s in range(8):
                            pj = psA.tile([128, 512], f32, tag="pj")
                            for cc in range(4):
                                nc.tensor.matmul(
                                    pj,
                                    wt[:, cc, h * HD:(h + 1) * HD],
                                    xr_t[cc][:, s * 512:(s + 1) * 512],
                                    start=(cc == 0), stop=(cc == 3))
                            nc.vector.tensor_scalar(
                                out=osb[:, s * 512:(s + 1) * 512], in0=pj,
                                scalar1=bt[:, h, :], scalar2=None, op0=OP.add)

                # vT: transpose v 128x128 blocks
                for h in range(2):
                    for kc in range(NKC):
                        pv = psAs.tile([128, 128], bf16, tag="pvt")
                        nc.tensor.transpose(
                            pv, v_sb[h][:, kc * 128:(kc + 1) * 128], ident_bf)
                        nc.vector.tensor_copy(vT_sb[h][:, kc, :], pv)

            # ---------------- Phase B: attention + output projection ----------------
            with tc.tile_pool(name="ptp", bufs=PT_BUFS) as ptp, \
                 tc.tile_pool(name="unp", bufs=2) as unp, \
                 tc.tile_pool(name="mb", bufs=2) as mb, \
                 tc.tile_pool(name="psS", bufs=2, space="PSUM") as psS, \
                 tc.tile_pool(name="psU", bufs=2, space="PSUM") as psU, \
                 tc.tile_pool(name="psD", bufs=2, space="PSUM") as psD:

                pending = []
                un_tiles = [None, None]
                for i in range(17):
                    if i < 16:
                        h, g = i % 2, i // 2
                        qg = q_sb[h][:, g * 512:(g + 1) * 512]
                        cur = []
                        for kcp in range(16):
                            ps_s = psS.tile([128, 2, 512], f32, tag="s")
                            for j in range(2):
                                kc = kcp * 2 + j
                                nc.tensor.matmul(
                                    ps_s[:, j, :],
                                    k_sb[h][:, kc * 128:(kc + 1) * 128],
                                    qg, start=True, stop=True)
                            pt = ptp.tile([128, 2, 512], bf16, tag="pt")
                            nc.scalar.activation(out=pt, in_=ps_s, func=AF.Exp,
                                                 scale=SCALE)
                            cur.append(pt)
                        pending.append((h, g, cur))
                    if len(pending) > (1 if i < 16 else 0):
                        h2, g2, pts = pending.pop(0)
                        U = psU.tile([128, 512], f32, tag="u")
                        D = psD.tile([128, 512], f32, tag="dp")
                        for kcp in range(16):
                            for j in range(2):
                                kc = kcp * 2 + j
                                nc.tensor.matmul(U, vT_sb[h2][:, kc, :],
                                                 pts[kcp][:, j, :],
                                                 start=(kc == 0), stop=(kc == 31))
                        for kcp in range(16):
                            for j in range(2):
                                kc = kcp * 2 + j
                                nc.tensor.matmul(D, ones_bf, pts[kcp][:, j, :],
                                                 start=(kc == 0), stop=(kc == 31))
                        recD = mb.tile([128, 512], f32, tag="recd")
                        nc.vector.reciprocal(recD, D)
                        un = unp.tile([128, 512], f32r, tag=f"un{h2}")
                        nc.vector.tensor_mul(un, U, recD)
                        un_tiles[h2] = un
                        if h2 == 1:
                            ost = mb.tile([128, 4, 512], f32, tag="ost")
                            for m in range(4):
                                pp = psD.tile([128, 512], f32, tag="dp")
                                for hh in range(2):
                                    nc.tensor.matmul(
                                        pp, wp_t[:, hh, m * 128:(m + 1) * 128],
                                        un_tiles[hh],
                                        start=(hh == 0), stop=(hh == 1))
                                nc.vector.tensor_copy(ost[:, m, :], pp)
                            nc.sync.dma_start(
                                out=out_d[:, g2 * 512:(g2 + 1) * 512]
                                    .rearrange("(m p) c -> p m c", p=128),
                                in_=ost)

    nc.compile()
    return nc


def _get_program():
    import concourse  # noqa: F401  (ensure import works before caching)
    global _PROGRAM
    try:
        return _PROGRAM
    except NameError:
        _PROGRAM = _build_program()
        return _PROGRAM


def _host_prep(inputs):
    x = np.ascontiguousarray(np.asarray(inputs["x"], dtype=np.float32))
    wq = np.asarray(inputs["wq"], dtype=np.float32)
    wk = np.asarray(inputs["wk"], dtype=np.float32)
    wv = np.asarray(inputs["wv"], dtype=np.float32)
    wp = np.asarray(inputs["wp"], dtype=np.float32)
    gnw = np.asarray(inputs["gn_w"], dtype=np.float32).reshape(4, 128, 1)
    gnb = np.asarray(inputs["gn_b"], dtype=np.float32).reshape(4, 128, 1)
    bq = np.asarray(inputs["bq"], dtype=np.float32)
    bk = np.asarray(inputs["bk"], dtype=np.float32)
    bv = np.asarray(inputs["bv"], dtype=np.float32)
    gt = np.zeros((C, NG), dtype=np.float32)
    gt[np.arange(C), np.arange(C) // (C // NG)] = 1.0 / (C // NG)
    ex = np.zeros((NG, 4, 128), dtype=np.float32)
    for t in range(4):
        cl = np.arange(128)
        ex[8 * t + cl // 16, t, cl] = 1.0

    in_maps = []
    for core in range(N_CORES):
        b, p = core // 2, core % 2
        ch0 = 2 * HD * p
        in_maps.append({
            "x": np.ascontiguousarray(x[b].reshape(C, HW)),
            "wqT": np.ascontiguousarray(wq[ch0:ch0 + 2 * HD, :].T),
            "wkT": np.ascontiguousarray(wk[ch0:ch0 + 2 * HD, :].T),
            "wvT": np.ascontiguousarray(wv[ch0:ch0 + 2 * HD, :].T),
            "wpT": np.ascontiguousarray(wp[:, ch0:ch0 + 2 * HD].T),
            "gnw": gnw, "gnb": gnb,
            "bq": np.ascontiguousarray(bq[ch0:ch0 + 2 * HD].reshape(2, 128, 1)),
            "bk": np.ascontiguousarray(bk[ch0:ch0 + 2 * HD].reshape(2, 128, 1)),
            "bv": np.ascontiguousarray(bv[ch0:ch0 + 2 * HD].reshape(2, 128, 1)),
            "gt": gt, "ex": ex,
        })
    return x, in_maps


def kernel(**inputs):
    from concourse.bass_utils import run_bass_kernel_spmd
    x, in_maps = _host_prep(inputs)
    bp = np.asarray(inputs["bp"], dtype=np.float32)
    nc = _get_program()
    res = run_bass_kernel_spmd(nc, in_maps, core_ids=list(range(N_CORES)))
    parts = [res.results[c]["out_part"] for c in range(N_CORES)]
    out = np.empty((B, C, HW), dtype=np.float32)
    for b in range(B):
        out[b] = (x[b].reshape(C, HW) + bp[:, None]
                  + parts[2 * b] + parts[2 * b + 1])
    return out.reshape(B, C, H, W)
